# revision 1
# baseline (speedup 1.0000x reference)
"""TRN2 Bass kernel for nn_MultiHeadAttention (B=4, S=2048, D=1024, H=16, DH=64).

Sharding (8 cores): core c -> batch b = c//2, head-half hh = c%2 (8 heads each).

Per-core structure (single TileContext; phases overlap via data deps):
  - v projection (all 8 heads) in natural [s, dh] layout with a ones column
    per head (V_aug [s, 65]) so the PV matmul also yields softmax denominators.
  - pair loop p=0..3: q/k projections for head pair p (pair-stacked
    [128 = 2x64(dh), S], from host-pre-transposed xT so the contraction dim d
    sits on partitions), then attention for the pair's 2 heads.  The Tile
    scheduler overlaps pair p's attention (ACT-bound exp) with pair p+1's
    projections (PE) since they touch disjoint buffers.
  - attention: scores computed TRANSPOSED sT[sk, sq] = kT.T @ qT (stationary
    kT [64, 128-sk-tile], moving qT [64, 512]), two 512-wide score matmuls
    share one 1024-wide exp (halves ACT instruction overhead; no max
    subtraction: scores/8 ~ N(0,1), fp32 exp is safe), then
    ctxT[dh, sq] += V_aug.T @ expT accumulated over sk tiles.  Row 64 of
    ctxT_aug = sum(exp); its reciprocal is partition-broadcast with a step-0
    SBUF->SBUF DMA and multiplied in on the DVE.
  - out-projection: out[s, :] = sum_p ctxT_chunk.T @ Wo_chunk (+ bo/2 so the
    host-side pairwise sum adds bo exactly once).
Host: out[b] = core(2b) partial + core(2b+1) partial.

All matmuls run in float32r (TF32-class precision, 1 cycle/row at N>=256 on
TRN2 vs 4 for fp32).  This walrus build accepts only ONE sync-wait command per
instruction, so after TileContext scheduling we split extra waits into
single-wait NoOps on the same engine (legalize_waits).
"""

import sys

if "/opt/trn_rl_repo" not in sys.path:
    sys.path.insert(0, "/opt/trn_rl_repo")

import numpy as np

import concourse.bass as bass
import concourse.mybir as mybir
import concourse.tile as tile
from concourse.bass_utils import run_bass_kernel_spmd

F32 = mybir.dt.float32
F32R = mybir.dt.float32r
EXP = mybir.ActivationFunctionType.Exp

B, S_FULL, D, H = 4, 2048, 1024, 16
DH = 64
NCORES = 8


def legalize_waits(nc, max_waits=1):
    """Split >max_waits sync-waits per instruction into single-wait NoOps on
    the same engine, placed immediately before (per-engine order preserved)."""
    n = 0
    for fn in nc.m.functions:
        for blk in fn.blocks:
            out = []
            for inst in blk.instructions:
                si = inst.sync_info
                if si is not None and len(si.on_wait) > max_waits:
                    waits = list(si.on_wait)
                    for w in waits[:-max_waits]:
                        nop = mybir.InstNoOp(
                            name=f"WSPLIT-{n}", ins=[], outs=[], engine=inst.engine
                        )
                        n += 1
                        nop.sync_info = mybir.SyncInfo(on_wait=[w], on_update=[])
                        out.append(nop)
                    inst.sync_info = mybir.SyncInfo(
                        on_wait=waits[-max_waits:], on_update=list(si.on_update)
                    )
                out.append(inst)
            blk.instructions[:] = out
    return n


def _bcast_ap(src_ap, parts=128):
    """Partition-broadcast a [1, N] AP to [parts, N] via a step-0 dim."""
    return bass.AP(
        tensor=src_ap.tensor,
        offset=src_ap.offset,
        ap=[[0, parts], list(src_ap.ap[-1])],
    )


def build_nc(S=S_FULL, legalize=True):
    NQB = S // 1024  # 1024-wide sq blocks
    NST = S // 128   # sk tiles
    NSB = S // 512   # 512-wide s blocks (projection granularity)
    nc = bass.Bass()
    xT_d = nc.dram_tensor("xt", [D, S], F32R, kind="ExternalInput")
    wq_d = nc.dram_tensor("wq", [128, 4096], F32R, kind="ExternalInput")
    wk_d = nc.dram_tensor("wk", [128, 4096], F32R, kind="ExternalInput")
    wv_d = nc.dram_tensor("wv", [128, 4096], F32R, kind="ExternalInput")
    wo_d = nc.dram_tensor("wo", [128, 4096], F32R, kind="ExternalInput")
    bqk_d = nc.dram_tensor("bqk", [128, 8], F32, kind="ExternalInput")
    bv_d = nc.dram_tensor("bv", [1, 512], F32, kind="ExternalInput")
    bo_d = nc.dram_tensor("bo", [1, 1024], F32, kind="ExternalInput")
    vinit_d = nc.dram_tensor("vinit", [1, NST * 520], F32R, kind="ExternalInput")
    ones_d = nc.dram_tensor("ones", [1, 64], F32R, kind="ExternalInput")
    out_d = nc.dram_tensor("out", [S, 1024], F32, kind="ExternalOutput")

    with tile.TileContext(nc) as tc, nc.allow_low_precision(
        reason="f32r (tf32-class) matmul inputs are intentional"
    ):
        with tc.tile_pool(name="persist", bufs=1) as pp, \
             tc.tile_pool(name="psP", bufs=2, space="PSUM") as psP, \
             tc.tile_pool(name="psS", bufs=2, space="PSUM") as psS, \
             tc.tile_pool(name="psX", bufs=2, space="PSUM") as psX:
            qT = pp.tile([128, 4 * S], F32R)
            kT = pp.tile([128, 4 * S], F32R)
            vall = pp.tile([128, NST * 520], F32R)  # per s-tile: 8 heads x 65
            bqk = pp.tile([128, 8], F32)
            bv_b = pp.tile([128, 512], F32)
            bo_b = pp.tile([128, 1024], F32)
            ones = pp.tile([1, 64], F32R)

            # ---- projection sweep 1: pair 0 q/k + all of v ----
            with tc.tile_pool(name="w2p", bufs=1) as w2p:
                # pairs 1-3 weight columns, resident through attention
                wq2 = w2p.tile([128, 3072], F32R)
                wk2 = w2p.tile([128, 3072], F32R)

                with tc.tile_pool(name="aw", bufs=1) as aw, \
                     tc.tile_pool(name="xp", bufs=1) as xp:
                    wq0 = aw.tile([128, 1024], F32R)
                    wk0 = aw.tile([128, 1024], F32R)
                    wv = aw.tile([128, 4096], F32R)

                    def load_x(pool, sb, gen):
                        xs = []
                        for ch in range(8):
                            xt = pool.tile([128, 512], F32R, tag=f"x{ch}",
                                           name=f"x{ch}_{gen}_{sb}")
                            nc.sync.dma_start(
                                out=xt,
                                in_=xT_d[ch * 128:(ch + 1) * 128,
                                         sb * 512:(sb + 1) * 512],
                            )
                            xs.append(xt)
                        return xs

                    def qk_group(wmat, nch, wcol0, dstT, bcol, p, sb, xs):
                        stride = wmat.shape[1] // 8
                        ps_q = psP.tile([128, 512], F32, tag="pp", name="ps_q")
                        for ch in range(8):
                            nc.tensor.matmul(
                                ps_q,
                                wmat[:, wcol0 + ch * stride:
                                     wcol0 + ch * stride + 128],
                                xs[ch],
                                start=(ch == 0),
                                stop=(ch == 7),
                            )
                        nc.vector.tensor_scalar_add(
                            dstT[:, p * S + sb * 512: p * S + (sb + 1) * 512],
                            ps_q,
                            bqk[:, bcol + p: bcol + p + 1],
                        )

                    # DMA issue order: first-needed bytes first
                    xs0 = load_x(xp, 0, 1)
                    for ch in range(8):  # pair-0 columns: 1MB total
                        nc.sync.dma_start(
                            out=wq0[:, ch * 128:(ch + 1) * 128],
                            in_=wq_d[:, ch * 512: ch * 512 + 128])
                        nc.sync.dma_start(
                            out=wk0[:, ch * 128:(ch + 1) * 128],
                            in_=wk_d[:, ch * 512: ch * 512 + 128])
                    nc.sync.dma_start(out=bqk, in_=bqk_d[:, :])
                    nc.sync.dma_start(out=ones, in_=ones_d[:, :])
                    for ch in range(8):
                        nc.sync.dma_start(
                            out=wv[:, ch * 512:(ch + 1) * 512],
                            in_=wv_d[:, ch * 512:(ch + 1) * 512])
                    nc.sync.dma_start(out=bv_b, in_=_bcast_ap(bv_d[:, :]))
                    # V_aug template (1.0 in each head's 65th col)
                    nc.sync.dma_start(out=vall, in_=_bcast_ap(vinit_d[:, :]))
                    nc.sync.dma_start(out=bo_b, in_=_bcast_ap(bo_d[:, :]))

                    for sb in range(NSB):
                        xs = xs0 if sb == 0 else load_x(xp, sb, 1)
                        qk_group(wq0, 8, 0, qT, 0, 0, sb, xs)
                        qk_group(wk0, 8, 0, kT, 4, 0, sb, xs)
                        for t4 in range(4):
                            st = sb * 4 + t4
                            ps_v = psP.tile([128, 512], F32, tag="pp", name="ps_v")
                            for ch in range(8):
                                nc.tensor.matmul(
                                    ps_v,
                                    xs[ch][:, t4 * 128:(t4 + 1) * 128],
                                    wv[:, ch * 512:(ch + 1) * 512],
                                    start=(ch == 0),
                                    stop=(ch == 7),
                                )
                            dst = vall[:, st * 520:(st + 1) * 520].rearrange(
                                "p (h e) -> p h e", e=65
                            )[:, :, 0:64]
                            nc.vector.tensor_add(
                                dst,
                                ps_v.rearrange("p (h e) -> p h e", e=64),
                                bv_b.rearrange("p (h e) -> p h e", e=64),
                            )

                for ch in range(8):  # pairs 1-3 columns, needed from block 0 st=5
                    nc.sync.dma_start(
                        out=wq2[:, ch * 384:(ch + 1) * 384],
                        in_=wq_d[:, ch * 512 + 128:(ch + 1) * 512])
                    nc.sync.dma_start(
                        out=wk2[:, ch * 384:(ch + 1) * 384],
                        in_=wk_d[:, ch * 512 + 128:(ch + 1) * 512])

                # ---- attention; pairs 1-3 projections interleaved ----
                with tc.tile_pool(name="bc", bufs=1) as bc:
                  ctxT = bc.tile([128, 4 * S], F32R)
                  with tc.tile_pool(name="p2", bufs=1) as p2, \
                       tc.tile_pool(name="at", bufs=5) as atp, \
                       tc.tile_pool(name="sm", bufs=3) as sm:

                    def pass2_gen():
                        for p in (1, 2, 3):
                            for sb in range(NSB):
                                xs = load_x(p2, sb, 1 + p)
                                qk_group(wq2, 8, (p - 1) * 128, qT, 0, p, sb, xs)
                                yield
                                qk_group(wk2, 8, (p - 1) * 128, kT, 4, p, sb, xs)
                                yield

                    pass2 = pass2_gen()

                    for h in range(8):
                        p = h // 2
                        r0 = 64 * (h % 2)
                        for qb in range(NQB):
                            ps_c = [psX.tile([65, 512], F32, tag="pctx",
                                             name=f"ps_c{_i}")
                                    for _i in range(2)]
                            def emit_pv(st, at):
                                for half in range(2):
                                    nc.tensor.matmul(
                                        ps_c[half],
                                        vall[:, st * 520 + h * 65:
                                             st * 520 + (h + 1) * 65],
                                        at[:, half * 512:(half + 1) * 512],
                                        start=(st == 0),
                                        stop=(st == NST - 1),
                                    )

                            pv_pending = None
                            for st in range(NST):
                                if h < 6 and st in (NST - 2, NST - 1):
                                    next(pass2, None)
                                ps_s = psS.tile([128, 1024], F32, tag="ps")
                                for half in range(2):
                                    nc.tensor.matmul(
                                        ps_s[:, half * 512:(half + 1) * 512],
                                        kT[r0:r0 + 64,
                                           p * S + st * 128: p * S + (st + 1) * 128],
                                        qT[r0:r0 + 64,
                                           p * S + qb * 1024 + half * 512:
                                           p * S + qb * 1024 + (half + 1) * 512],
                                        start=True,
                                        stop=True,
                                    )
                                at = atp.tile([128, 1024], F32R, tag="at")
                                nc.scalar.activation(at, ps_s, EXP, scale=0.125)
                                if pv_pending is not None:
                                    emit_pv(*pv_pending)
                                pv_pending = (st, at)
                            emit_pv(*pv_pending)
                            for half in range(2):
                                rsum = sm.tile([1, 512], F32R, tag="rsum")
                                nc.vector.reciprocal(rsum, ps_c[half][64:65, :])
                                ps_b = psP.tile([64, 512], F32, tag="pp",
                                                name="ps_b")
                                nc.tensor.matmul(ps_b, ones, rsum,
                                                 start=True, stop=True)
                                rb = sm.tile([64, 512], F32, tag="rb")
                                nc.vector.tensor_copy(rb, ps_b)
                                c0 = p * S + qb * 1024 + half * 512
                                nc.vector.tensor_mul(
                                    ctxT[r0:r0 + 64, c0:c0 + 512],
                                    ps_c[half][0:64, :],
                                    rb,
                                )
                    for _ in pass2:
                        pass

                  # ---- out projection ----
                  with tc.tile_pool(name="co", bufs=1) as co, \
                       tc.tile_pool(name="cot", bufs=3) as cot:
                      wo = co.tile([128, 4096], F32R)
                      nc.sync.dma_start(out=wo, in_=wo_d[:, :])
                      for t in range(NST):
                          ps_o = psS.tile([128, 1024], F32, tag="ps", name="ps_o")
                          for p in range(4):
                              lhsT = ctxT[:, p * S + t * 128: p * S + (t + 1) * 128]
                              for half in range(2):
                                  nc.tensor.matmul(
                                      ps_o[:, half * 512:(half + 1) * 512],
                                      lhsT,
                                      wo[:, p * 1024 + half * 512:
                                         p * 1024 + (half + 1) * 512],
                                      start=(p == 0),
                                      stop=(p == 3),
                                  )
                          ot = cot.tile([128, 1024], F32, tag="ot")
                          nc.vector.tensor_add(ot, ps_o, bo_b)
                          nc.sync.dma_start(out=out_d[t * 128:(t + 1) * 128, :], in_=ot)

    if legalize:
        legalize_waits(nc)
    return nc


def pack_core_inputs(c, x, Wq, bq, Wk, bk, Wv, bv, Wo, bo, S=S_FULL):
    """Pack full-model inputs into core c's device tensors."""
    b = c // 2
    hh = c % 2
    hs = slice(hh * 8, hh * 8 + 8)

    def pack_w(W):  # [8, D, DH] -> [128, 4096]: free = chunk*512 + (h*64+dh)
        W2 = np.transpose(W, (1, 0, 2)).reshape(D, 512)      # [d, h*dh]
        return np.ascontiguousarray(
            np.transpose(W2.reshape(8, 128, 512), (1, 0, 2)).reshape(128, 4096)
        )

    xT = np.ascontiguousarray(x[b].T)                         # [D, S]
    wq = pack_w(Wq[hs])
    wk = pack_w(Wk[hs])
    wv = pack_w(Wv[hs])
    # Wo rows for this half's features: [512, 1024] -> [128, 4*1024]
    Wr = Wo[hh * 512:(hh + 1) * 512]
    wo = np.ascontiguousarray(
        np.transpose(Wr.reshape(4, 128, 1024), (1, 0, 2)).reshape(128, 4096)
    )
    bqk = np.concatenate(
        [bq[hs].reshape(4, 128).T, bk[hs].reshape(4, 128).T], axis=1
    )                                                         # [128, 8]
    bvp = bv[hs].reshape(1, 512)
    bop = (0.5 * bo).reshape(1, 1024)
    NST = S // 128
    vinit = np.zeros((1, NST * 520), dtype=np.float32)
    vinit[0, 64::65] = 1.0
    return {
        "vinit": vinit,
        "ones": np.ones((1, 64), dtype=np.float32),
        "xt": xT.astype(np.float32),
        "wq": wq.astype(np.float32),
        "wk": wk.astype(np.float32),
        "wv": wv.astype(np.float32),
        "wo": wo.astype(np.float32),
        "bqk": np.ascontiguousarray(bqk).astype(np.float32),
        "bv": bvp.astype(np.float32),
        "bo": bop.astype(np.float32),
    }


_NC_CACHE = {}


def _get_nc(S=S_FULL):
    if S not in _NC_CACHE:
        _NC_CACHE[S] = build_nc(S)
    return _NC_CACHE[S]


def kernel(x, Wq, bq, Wk, bk, Wv, bv, Wo, bo, _trace=False):
    x, Wq, bq, Wk, bk, Wv, bv, Wo, bo = (
        np.asarray(a, dtype=np.float32) for a in (x, Wq, bq, Wk, bk, Wv, bv, Wo, bo)
    )
    nc = _get_nc()
    in_maps = [
        pack_core_inputs(c, x, Wq, bq, Wk, bk, Wv, bv, Wo, bo) for c in range(NCORES)
    ]
    res = run_bass_kernel_spmd(nc, in_maps, list(range(NCORES)), trace=_trace)
    out = np.empty((B, S_FULL, D), dtype=np.float32)
    for b in range(B):
        out[b] = res.results[2 * b]["out"] + res.results[2 * b + 1]["out"]
    if _trace:
        kernel.last_results = res
    return out



# revision 7
# speedup vs baseline: 1.2618x; 1.2618x over previous
"""TRN2 Bass kernel for nn_MultiHeadAttention (B=4, S=2048, D=1024, H=16, DH=64).

Sharding (8 cores): core c -> batch b = c//2, head-half hh = c%2 (8 heads each).
Host sums the two per-core partial out-projections per batch.

All matmul operands are bf16 (1 cycle/row at any N, vs f32r's N>=256
requirement); accumulation stays f32 in PSUM.  Measured end-to-end rel err
~5.5e-3 vs the f32 reference (gate 2e-2).

Structure (single TileContext, one fluid phase; x resident in SBUF):
  - scores TRANSPOSED sT[sk, sq] = kT.T @ qT (stationary kT [64, 128-sk],
    moving qT [64, 512]); one 1024-wide exp per sk-tile -> at [128sk, 1024sq]
    bf16.
  - PV reoriented: at tile is STATIONARY [sk 128, sq 128-chunk], V_aug
    [sk 128, 65] is MOVING (N=65; full K=M=128) -> ctx accumulates in natural
    [sq, 65] layout, 65th column = softmax denominator on the sq partition.
    This halves PV's PE cycles vs streaming at with N=512 at M=65.
  - normalization: per-partition reciprocal + tensor_scalar_mul on DVE (no
    PE broadcast matmuls); pairs of heads share a ctx_nat2 [sq 128, 128]
    buffer which one XBAR DMA transpose flips into ctxT [128 feats, sq] for
    the out-projection.
  - v-projection computes V_aug [sk, 65-per-head] with a memset-initialized
    ones column (no 4MB DMA); split into head-half units for fine scheduling.
  - the emission order interleaves projections/v/out-proj chunks into the
    ACT-bound attention stream as PE filler, keeping both PE (~274us) and
    ACT (~266us) near-continuously busy.

This walrus build accepts only ONE sync-wait per instruction, so after
TileContext scheduling extra waits are split into single-wait NoOps on the
same engine (legalize_waits).
"""

import sys

if "/opt/trn_rl_repo" not in sys.path:
    sys.path.insert(0, "/opt/trn_rl_repo")

import numpy as np
import ml_dtypes

import concourse.bass as bass
import concourse.mybir as mybir
import concourse.tile as tile
from concourse.bass_utils import run_bass_kernel_spmd

F32 = mybir.dt.float32
BF16 = mybir.dt.bfloat16
EXP = mybir.ActivationFunctionType.Exp

B, S_FULL, D, H = 4, 2048, 1024, 16
DH = 64
NCORES = 8


def legalize_waits(nc, max_waits=1):
    """Split >max_waits sync-waits per instruction into single-wait NoOps on
    the same engine, placed immediately before (per-engine order preserved)."""
    n = 0
    for fn in nc.m.functions:
        for blk in fn.blocks:
            out = []
            for inst in blk.instructions:
                si = inst.sync_info
                if si is not None and len(si.on_wait) > max_waits:
                    waits = list(si.on_wait)
                    for w in waits[:-max_waits]:
                        nop = mybir.InstNoOp(
                            name=f"WSPLIT-{n}", ins=[], outs=[], engine=inst.engine
                        )
                        n += 1
                        nop.sync_info = mybir.SyncInfo(on_wait=[w], on_update=[])
                        out.append(nop)
                    inst.sync_info = mybir.SyncInfo(
                        on_wait=waits[-max_waits:], on_update=list(si.on_update)
                    )
                out.append(inst)
            blk.instructions[:] = out
    return n


def _bcast_ap(src_ap, parts=128):
    """Partition-broadcast a [1, N] AP to [parts, N] via a step-0 dim."""
    return bass.AP(
        tensor=src_ap.tensor,
        offset=src_ap.offset,
        ap=[[0, parts], list(src_ap.ap[-1])],
    )


def build_nc(S=S_FULL, legalize=True):
    NQB = S // 1024  # 1024-wide sq blocks
    NST = S // 128   # sk tiles
    NSB = S // 512   # 512-wide s blocks (projection granularity)
    nc = bass.Bass()
    xT_d = nc.dram_tensor("xt", [D, S], BF16, kind="ExternalInput")
    wq_d = nc.dram_tensor("wq", [128, 4096], BF16, kind="ExternalInput")
    wk_d = nc.dram_tensor("wk", [128, 4096], BF16, kind="ExternalInput")
    wv_d = nc.dram_tensor("wv", [128, 4096], BF16, kind="ExternalInput")
    wo_d = nc.dram_tensor("wo", [128, 4096], BF16, kind="ExternalInput")
    bqk_d = nc.dram_tensor("bqk", [128, 8], F32, kind="ExternalInput")
    bv_d = nc.dram_tensor("bv", [1, 512], F32, kind="ExternalInput")
    bo_d = nc.dram_tensor("bo", [1, 1024], F32, kind="ExternalInput")
    out_d = nc.dram_tensor("out", [S, 1024], BF16, kind="ExternalOutput")

    with tile.TileContext(nc) as tc, nc.allow_low_precision(
        reason="bf16 matmul operands are intentional; f32 accumulate"
    ):
        with tc.tile_pool(name="persist", bufs=1) as pp, \
             tc.tile_pool(name="psS", bufs=2, space="PSUM") as psS, \
             tc.tile_pool(name="psC", bufs=1, space="PSUM") as psC, \
             tc.tile_pool(name="psP", bufs=2, space="PSUM") as psP, \
             tc.tile_pool(name="atp", bufs=12) as atp, \
             tc.tile_pool(name="cn2", bufs=2) as cn2, \
             tc.tile_pool(name="rzp", bufs=4) as rzp, \
             tc.tile_pool(name="otp", bufs=3) as otp:
            xres = pp.tile([128, 8 * S], BF16)          # d-chunk ch at cols ch*S
            qT = pp.tile([128, 4 * S], BF16)
            kT = pp.tile([128, 4 * S], BF16)
            vall = pp.tile([128, NST * 520], BF16)      # per st: 8 heads x 65
            ctxT = pp.tile([128, 4 * S], BF16)
            wq0 = pp.tile([128, 1024], BF16)
            wk0 = pp.tile([128, 1024], BF16)
            wq2 = pp.tile([128, 3072], BF16)
            wk2 = pp.tile([128, 3072], BF16)
            wv = pp.tile([128, 4096], BF16)
            wo = pp.tile([128, 4096], BF16)
            bqk = pp.tile([128, 8], F32)
            bv_b = pp.tile([128, 512], F32)
            bo_b = pp.tile([128, 1024], F32)

            # ---- DMA issue order: first-needed bytes first ----
            def dma_w_pair0(dst, src_d):
                # pair-0 columns [ch*512 : ch*512+128] for ch in 0..7
                src = src_d[:, :].rearrange("p (c w) -> p c w", w=512)[:, :, 0:128]
                nc.sync.dma_start(out=dst.rearrange("p (c w) -> p c w", w=128),
                                  in_=src)

            def dma_w_rest(dst, src_d):
                # pairs 1-3 columns [ch*512+128 : (ch+1)*512]
                src = src_d[:, :].rearrange("p (c w) -> p c w", w=512)[:, :, 128:512]
                nc.sync.dma_start(out=dst.rearrange("p (c w) -> p c w", w=384),
                                  in_=src)

            def dma_x(sb):
                for ch in range(8):
                    nc.sync.dma_start(
                        out=xres[:, ch * S + sb * 512: ch * S + (sb + 1) * 512],
                        in_=xT_d[ch * 128:(ch + 1) * 128,
                                 sb * 512:(sb + 1) * 512],
                    )

            dma_w_pair0(wk0, wk_d)
            dma_x(0)
            dma_w_pair0(wq0, wq_d)
            nc.sync.dma_start(out=bqk, in_=bqk_d[:, :])
            dma_x(1)
            nc.sync.dma_start(out=wv, in_=wv_d[:, :])
            nc.sync.dma_start(out=bv_b, in_=_bcast_ap(bv_d[:, :]))
            dma_x(2)
            dma_x(3)
            dma_w_rest(wq2, wq_d)
            dma_w_rest(wk2, wk_d)
            nc.sync.dma_start(out=wo, in_=wo_d[:, :])
            nc.sync.dma_start(out=bo_b, in_=_bcast_ap(bo_d[:, :]))

            # ones columns of V_aug (col 64 of each 65-block)
            vones = vall.rearrange("p (x e) -> p x e", e=65)[:, :, 64:65]
            nc.vector.memset(vones, 1.0)

            # ---- emission helpers ----
            def xs(ch, sb):
                return xres[:, ch * S + sb * 512: ch * S + (sb + 1) * 512]

            def qk_unit(wmat, wcol0, dstT, bcol, p, sb):
                """One projection unit: 8 matmuls + bias add -> dstT cols."""
                stride = wmat.shape[1] // 8
                ps_q = psP.tile([128, 512], F32, tag="pp", name="ps_q")
                for ch in range(8):
                    nc.tensor.matmul(
                        ps_q,
                        wmat[:, wcol0 + ch * stride: wcol0 + ch * stride + 128],
                        xs(ch, sb),
                        start=(ch == 0),
                        stop=(ch == 7),
                    )
                nc.vector.tensor_scalar_add(
                    dstT[:, p * S + sb * 512: p * S + (sb + 1) * 512],
                    ps_q,
                    bqk[:, bcol + p: bcol + p + 1],
                )

            def v_unit(sb, t4, ph):
                """V_aug for one st tile, one head-half (4 heads)."""
                st = sb * 4 + t4
                ps_v = psP.tile([128, 256], F32, tag="pp", name="ps_v")
                for ch in range(8):
                    nc.tensor.matmul(
                        ps_v,
                        xs(ch, sb)[:, t4 * 128:(t4 + 1) * 128],
                        wv[:, ch * 512 + ph * 256: ch * 512 + (ph + 1) * 256],
                        start=(ch == 0),
                        stop=(ch == 7),
                    )
                dst = vall[:, st * 520 + ph * 260:
                           st * 520 + (ph + 1) * 260].rearrange(
                    "p (h e) -> p h e", e=65)[:, :, 0:64]
                nc.vector.tensor_add(
                    dst,
                    ps_v.rearrange("p (h e) -> p h e", e=64),
                    bv_b[:, ph * 256:(ph + 1) * 256].rearrange(
                        "p (h e) -> p h e", e=64),
                )

            def outproj_half(t, half, pool, tag):
                """out[t*128:(t+1)*128, half*512:...] = ctxT_t.T @ wo_half."""
                ps_o = pool.tile([128, 512], F32, tag=tag, name="ps_o")
                for p in range(4):
                    nc.tensor.matmul(
                        ps_o,
                        ctxT[:, p * S + t * 128: p * S + (t + 1) * 128],
                        wo[:, p * 1024 + half * 512: p * 1024 + (half + 1) * 512],
                        start=(p == 0),
                        stop=(p == 3),
                    )
                ot = otp.tile([128, 512], BF16, tag="ot")
                nc.vector.tensor_add(ot, ps_o, bo_b[:, half * 512:(half + 1) * 512])
                nc.sync.dma_start(
                    out=out_d[t * 128:(t + 1) * 128, half * 512:(half + 1) * 512],
                    in_=ot)

            # ---- attention block machinery ----
            def scores_exp(h, qb, st):
                p = h // 2
                r0 = 64 * (h % 2)
                ps_s = psS.tile([128, 1024], F32, tag="ps")
                for half in range(2):
                    nc.tensor.matmul(
                        ps_s[:, half * 512:(half + 1) * 512],
                        kT[r0:r0 + 64,
                           p * S + st * 128: p * S + (st + 1) * 128],
                        qT[r0:r0 + 64,
                           p * S + qb * 1024 + half * 512:
                           p * S + qb * 1024 + (half + 1) * 512],
                        start=True,
                        stop=True,
                    )
                at = atp.tile([128, 1024], BF16, tag="at")
                nc.scalar.activation(at, ps_s, EXP, scale=0.125)
                return at

            def pv(h, st, at, pc):
                """8 chunk matmuls: ctx[sq,65] += at_chunk.T-contracted V_aug."""
                pcA, pcB = pc
                for c in range(8):
                    dst = (pcA if c < 4 else pcB)
                    nc.tensor.matmul(
                        dst[:, (c % 4) * 65:(c % 4) * 65 + 65],
                        at[:, c * 128:(c + 1) * 128],
                        vall[:, st * 520 + h * 65: st * 520 + (h + 1) * 65],
                        start=(st == 0),
                        stop=(st == NST - 1),
                    )

            def norm_block(h, qb, pc, nat):
                """reciprocal + scale into ctx_nat2 half for this head."""
                pcA, pcB = pc
                hp = h % 2
                rz = rzp.tile([128, 8], F32, tag="rz")
                nc.vector.reciprocal(
                    rz[:, 0:4].rearrange("p (c o) -> p c o", o=1),
                    pcA.rearrange("p (c e) -> p c e", e=65)[:, :, 64:65])
                nc.vector.reciprocal(
                    rz[:, 4:8].rearrange("p (c o) -> p c o", o=1),
                    pcB.rearrange("p (c e) -> p c e", e=65)[:, :, 64:65])
                for c in range(8):
                    src = (pcA if c < 4 else pcB)
                    nc.vector.tensor_scalar_mul(
                        nat[c][:, hp * 64:(hp + 1) * 64],
                        src[:, (c % 4) * 65:(c % 4) * 65 + 64],
                        rz[:, c:c + 1],
                    )

            def transpose_pair(p, qb, nat):
                for c in range(8):
                    nc.sync.dma_start_transpose(
                        out=ctxT[:, p * S + qb * 1024 + c * 128:
                                 p * S + qb * 1024 + (c + 1) * 128],
                        in_=nat[c],
                    )

            # ---- filler queue (deadline order: pair1 by h2, v-half1 by h4's
            # PVs, pair2 by h4, pair3 by h6, q sb2-3 by qb1, outproj qb0
            # during qb1) ----
            from collections import deque
            fillers = deque()

            for sb in range(NSB):          # pair 1 k
                fillers.append(("qk", wk2, 0, kT, 4, 1, sb))
            for sb in (0, 1):              # pair 1 q (qb0)
                fillers.append(("qk", wq2, 0, qT, 0, 1, sb))
            # v head-half 1 interleaved with pair 2
            vq = [("v", sb, t4, 1) for sb in range(NSB) for t4 in range(4)]
            p2q = ([("qk", wk2, 128, kT, 4, 2, sb) for sb in range(NSB)]
                   + [("qk", wq2, 128, qT, 0, 2, sb) for sb in (0, 1)])
            while vq or p2q:
                for _ in range(3):
                    if vq:
                        fillers.append(vq.pop(0))
                if p2q:
                    fillers.append(p2q.pop(0))
            for sb in range(NSB):          # pair 3 k
                fillers.append(("qk", wk2, 256, kT, 4, 3, sb))
            for sb in (0, 1):              # pair 3 q (qb0)
                fillers.append(("qk", wq2, 256, qT, 0, 3, sb))
            for pr in range(4):            # q sb2-3 for qb=1
                wm, c0 = (wq0, 0) if pr == 0 else (wq2, (pr - 1) * 128)
                for sb in (2, 3):
                    fillers.append(("qk", wm, c0, qT, 0, pr, sb))
            for t in range(8):             # out-proj for qb0 tokens
                for half in range(2):
                    fillers.append(("op", t, half))

            def emit_filler(n=1):
                for _ in range(n):
                    if not fillers:
                        return
                    f = fillers.popleft()
                    if f[0] == "v":
                        v_unit(f[1], f[2], f[3])
                    elif f[0] == "qk":
                        qk_unit(f[1], f[2], f[3], f[4], f[5], f[6])
                    else:
                        outproj_half(f[1], f[2], psP, "pp")

            # ---- lead-in: pair0 k/q (sb0-1), h0 scores racing v ----
            qk_unit(wk0, 0, kT, 4, 0, 0)
            qk_unit(wq0, 0, qT, 0, 0, 0)
            qk_unit(wq0, 0, qT, 0, 0, 1)

            nat_pool = {}  # chunk -> ctx_nat2 tile, per pair (reallocated)

            def get_nat():
                return [cn2.tile([128, 128], BF16, tag=f"cn{c}",
                                 name=f"nat{c}") for c in range(8)]

            # h0 qb0 block, hand-interleaved with k0 sb1-3 + v head-half 0
            at_q = {}
            pcs = {}
            pcs[(0, 0)] = (psC.tile([128, 260], F32, tag="pcA", name="pcA00"),
                           psC.tile([128, 260], F32, tag="pcB", name="pcB00"))
            nat_pool[0] = get_nat()
            for st in range(4):
                at_q[st] = scores_exp(0, 0, st)
            for t4 in range(4):
                v_unit(0, t4, 0)
            qk_unit(wk0, 0, kT, 4, 0, 1)
            for st in range(4, 8):
                at_q[st] = scores_exp(0, 0, st)
            for st in range(4):
                pv(0, st, at_q.pop(st), pcs[(0, 0)])
            for t4 in range(4):
                v_unit(1, t4, 0)
            qk_unit(wk0, 0, kT, 4, 0, 2)
            for st in range(8, 12):
                at_q[st] = scores_exp(0, 0, st)
            for st in range(4, 8):
                pv(0, st, at_q.pop(st), pcs[(0, 0)])
            for t4 in range(4):
                v_unit(2, t4, 0)
            qk_unit(wk0, 0, kT, 4, 0, 3)
            for st in range(12, 16):
                at_q[st] = scores_exp(0, 0, st)
            for st in range(8, 12):
                pv(0, st, at_q.pop(st), pcs[(0, 0)])
            for t4 in range(4):
                v_unit(3, t4, 0)
            for st in range(12, 16):
                pv(0, st, at_q.pop(st), pcs[(0, 0)])
            norm_block(0, 0, pcs.pop((0, 0)), nat_pool[0])

            # ---- steady-state blocks ----
            def block(h, qb):
                p = h // 2
                pc = (psC.tile([128, 260], F32, tag="pcA", name=f"pcA{h}{qb}"),
                      psC.tile([128, 260], F32, tag="pcB", name=f"pcB{h}{qb}"))
                if h % 2 == 0:
                    nat_pool[p] = get_nat()
                nat = nat_pool[p]
                at_pend = []
                emit_filler(1)
                for st in range(NST):
                    at = scores_exp(h, qb, st)
                    at_pend.append((st, at))
                    if st in (2, 5, 8, 11, 13):
                        emit_filler(1)
                    if len(at_pend) >= 3:
                        s0, a0 = at_pend.pop(0)
                        pv(h, s0, a0, pc)
                for s0, a0 in at_pend:
                    pv(h, s0, a0, pc)
                norm_block(h, qb, pc, nat)
                if h % 2 == 1:
                    transpose_pair(p, qb, nat)

            for h in range(1, 8):
                block(h, 0)
            for h in range(0, 8):
                block(h, 1)

            # ---- tail: drain leftover fillers, then out-proj qb1 tokens ----
            emit_filler(len(fillers))
            for t in range(8, 16):
                for half in range(2):
                    outproj_half(t, half, psS, "ps")

    if legalize:
        legalize_waits(nc)
    return nc


def pack_core_inputs(c, x, Wq, bq, Wk, bk, Wv, bv, Wo, bo, S=S_FULL):
    """Pack full-model inputs into core c's device tensors (bf16)."""
    b = c // 2
    hh = c % 2
    hs = slice(hh * 8, hh * 8 + 8)
    bf = ml_dtypes.bfloat16

    def pack_w(W):  # [8, D, DH] -> [128, 4096]: free = chunk*512 + (h*64+dh)
        W2 = np.transpose(W, (1, 0, 2)).reshape(D, 512)      # [d, h*dh]
        return np.ascontiguousarray(
            np.transpose(W2.reshape(8, 128, 512), (1, 0, 2)).reshape(128, 4096)
        )

    xT = np.ascontiguousarray(x[b].T)                         # [D, S]
    wq = pack_w(Wq[hs])
    wk = pack_w(Wk[hs])
    wv = pack_w(Wv[hs])
    Wr = Wo[hh * 512:(hh + 1) * 512]
    wo = np.ascontiguousarray(
        np.transpose(Wr.reshape(4, 128, 1024), (1, 0, 2)).reshape(128, 4096)
    )
    bqk = np.concatenate(
        [bq[hs].reshape(4, 128).T, bk[hs].reshape(4, 128).T], axis=1
    )                                                         # [128, 8]
    bvp = bv[hs].reshape(1, 512)
    bop = (0.5 * bo).reshape(1, 1024)
    return {
        "xt": xT.astype(bf),
        "wq": wq.astype(bf),
        "wk": wk.astype(bf),
        "wv": wv.astype(bf),
        "wo": wo.astype(bf),
        "bqk": np.ascontiguousarray(bqk).astype(np.float32),
        "bv": bvp.astype(np.float32),
        "bo": bop.astype(np.float32),
    }


_NC_CACHE = {}


def _get_nc(S=S_FULL):
    if S not in _NC_CACHE:
        _NC_CACHE[S] = build_nc(S)
    return _NC_CACHE[S]


def kernel(x, Wq, bq, Wk, bk, Wv, bv, Wo, bo, _trace=False):
    x, Wq, bq, Wk, bk, Wv, bv, Wo, bo = (
        np.asarray(a, dtype=np.float32) for a in (x, Wq, bq, Wk, bk, Wv, bv, Wo, bo)
    )
    nc = _get_nc()
    in_maps = [
        pack_core_inputs(c, x, Wq, bq, Wk, bk, Wv, bv, Wo, bo) for c in range(NCORES)
    ]
    res = run_bass_kernel_spmd(nc, in_maps, list(range(NCORES)), trace=_trace)
    out = np.empty((B, S_FULL, D), dtype=np.float32)
    for b in range(B):
        out[b] = (res.results[2 * b]["out"].astype(np.float32)
                  + res.results[2 * b + 1]["out"].astype(np.float32))
    if _trace:
        kernel.last_results = res
    return out
